# revision 1
# baseline (speedup 1.0000x reference)
"""Trainium2 Bass kernel for nn_CPCircuitLayer_63350767616542 (embedding_lookup).

Reference math:
    seq_emb = einsum("bsh,rh->bsr", hidden_states, W_seq)          # [B,S,R]
    hid_fac = hidden_embeddings * cp_weight[0][None, :]            # [H,R]
    out[b,n] = sum_r seq_emb[b, si[n], r] * hid_fac[hi[n], r]      # [B,N]
    return out.reshape(B, S, N // S)

all_indices is the row-major cartesian product of (seq_idx, hidden_idx), so the
gather is the identity and the whole layer collapses to a two-matmul chain:
    P = hidden_states @ W_seq.T @ hid_fac.T                        # [B,S,H]
A host-side fallback gather handles any non-cartesian index list.

Sharding: flatten (B,S) -> 2048 rows, shard rows across the 8 cores (256 rows
per core, data-parallel, no collectives). Each core splits its rows into two
128-row chunks m and computes, per chunk,
    tt[m] = W_seq @ X_m^T            ([64, 128], 4 accumulating k-matmuls)
    O_m   = tt[m]^T @ hid_fac^T      ([128, 512])
with bf16 operands AND bf16 output (host converts back to f32): ~4e-3 rel
err, half the DMA bytes on both sides and full-rate PE.

Device schedule (raw bass, hand-scheduled):
    SP:   input DMAs [W|X0|X1] and [h] (the h image rides its own small DMA:
          it has 64 partitions, incompatible with the 128-partition X image),
          then the two output DMAs as their col-split copies complete.
    PE:   mm1[0] / mm1[1] as soon as [W|X] lands, mm2[m] once tt[m] is
          staged in SBUF (bf16) and h has landed.
    DVE:  tt[0] PSUM->SBUF(bf16), then left col-slices of the output copies.
    Act:  tt[1] PSUM->SBUF(bf16), then right col-slices of the output copies.
"""

import os

import numpy as np

B, S, H, R = 2, 1024, 512, 64
N_CORES = 8
ROWS = B * S                      # 2048 flattened rows
RPC = ROWS // N_CORES             # 256 rows per core
KC = H // 128                     # 4 contraction chunks of 128
MC = RPC // 128                   # 2 output row chunks of 128
W_COLS = KC * R                   # 256 cols of the packed W image
XT_COLS = KC * 128                # 512 xt cols per row chunk
IMG_COLS = W_COLS + MC * XT_COLS  # 1280 cols of the packed [W|X0|X1] image

# Output-copy column split across the copy engines: DVE gets [0, c0), Act
# [c0, c1), Pool (gpsimd) [c1, H). Two boundaries -> 3-way split; one -> 2-way.
COPY_SPLITS = tuple(
    int(x) for x in os.environ.get("BASS_COPY_SPLITS", "352").split(",") if x
)
# mm2 piece issue order (indices into the DVE/Act/Pool piece list): engines
# whose copies finish last should get their piece first.
PIECE_PERM = tuple(
    int(x) for x in os.environ.get("BASS_PIECE_PERM", "0,1,2").split(",") if x
)
# Strip the per-engine RegisterMove/Drain preamble (zero + bounds-check regs,
# never read by this kernel's static DMAs): ~250ns off every engine's start.
STRIP_PREAMBLE = os.environ.get("BASS_STRIP_PREAMBLE", "1") == "1"

_cache = {}
LAST_RESULT = None                # BassKernelResults of the most recent run


def _get_nc():
    key = ("nc", COPY_SPLITS, PIECE_PERM, STRIP_PREAMBLE)
    if key in _cache:
        return _cache[key]

    import concourse.bass as bass
    import concourse.mybir as mybir

    f32 = mybir.dt.float32
    bf16 = mybir.dt.bfloat16

    nc = bass.Bass(
        "TRN2",
        target_bir_lowering=False,
        debug=False,
        num_devices=N_CORES,
    )

    xw_d = nc.dram_tensor("xw", [128, IMG_COLS], bf16, kind="ExternalInput")
    h_d = nc.dram_tensor("h", [R, H], bf16, kind="ExternalInput")
    out_d = [
        nc.dram_tensor(f"out{m}", [128, H], bf16, kind="ExternalOutput")
        for m in range(MC)
    ]

    with (
        nc.sbuf_tensor([128, IMG_COLS], bf16) as xw_sb,
        nc.sbuf_tensor([R, H], bf16) as h_sb,
        nc.sbuf_tensor([R, MC * 128], bf16) as tt_sb,
        nc.sbuf_tensor([128, H], bf16) as o0_sb,
        nc.sbuf_tensor([128, H], bf16) as o1_sb,
        nc.psum_tensor([R, MC * 128], f32) as tt_ps,
        nc.psum_tensor([128, H], f32) as o0_ps,
        nc.psum_tensor([128, H], f32) as o1_ps,
        nc.semaphore("s_d1") as s_d1,
        nc.semaphore("s_d2") as s_d2,
        nc.semaphore("s_h") as s_h,
        nc.semaphore("s_mm1") as s_mm1,
        nc.semaphore("s_tt") as s_tt,
        nc.semaphore("s_mm2") as s_mm2,
        nc.semaphore("s_oc0") as s_oc0,
        nc.semaphore("s_oc1") as s_oc1,
        nc.semaphore("s_out") as s_out,
        nc.Block(no_gpsimd_drain=True) as block,
    ):
        o_sb = [o0_sb, o1_sb]
        o_ps = [o0_ps, o1_ps]
        s_oc = [s_oc0, s_oc1]
        # per-chunk copy pieces [(c0, c1), ...] for DVE, Act, Pool in order
        bounds = (0,) + COPY_SPLITS + (H,)
        pieces = list(zip(bounds[:-1], bounds[1:]))
        N_COPY = len(pieces)
        perm = [p for p in PIECE_PERM if p < N_COPY]
        assert sorted(perm) == list(range(N_COPY))
        # sem count (1-based position in issue order) each engine waits for
        pos = [perm.index(e) + 1 for e in range(N_COPY)]

        def w_slice(k):
            return xw_sb[:, k * R : (k + 1) * R]

        def xt_slice(m, k):
            c0 = W_COLS + m * XT_COLS + k * 128
            return xw_sb[:, c0 : c0 + 128]

        @block.sync
        def _(sync):
            d1_cols = W_COLS + XT_COLS
            sync.dma_start(
                xw_sb[:, 0:d1_cols], xw_d.ap()[:, 0:d1_cols]
            ).then_inc(s_d1, 16)
            sync.dma_start(
                xw_sb[:, d1_cols:IMG_COLS], xw_d.ap()[:, d1_cols:IMG_COLS]
            ).then_inc(s_d2, 16)
            sync.dma_start(h_sb[:], h_d.ap()).then_inc(s_h, 16)
            for m in range(MC):
                sync.wait_ge(s_oc[m], N_COPY)
                sync.dma_start(out_d[m].ap(), o_sb[m][:]).then_inc(s_out, 16)
            sync.wait_ge(s_out, 16 * MC)

        @block.tensor
        def _(tensor):
            for m in range(MC):
                tensor.wait_ge(s_d1 if m == 0 else s_d2, 16)
                for k in range(KC):
                    mm = nc.tensor.matmul(
                        tt_ps[:, m * 128 : (m + 1) * 128],
                        w_slice(k),
                        xt_slice(m, k),
                        start=(k == 0),
                        stop=(k == KC - 1),
                    )
                mm.then_inc(s_mm1, 1)
            tensor.wait_ge(s_h, 16)
            # col-split mm2 so each copy engine's slice is ready sooner:
            # s_mm2 piece order per chunk matches the DVE/Act/Pool col split
            for m in range(MC):
                tensor.wait_ge(s_tt, m + 1)
                for pi in perm:
                    c0, c1 = pieces[pi]
                    nc.tensor.matmul(
                        o_ps[m][:, c0:c1],
                        tt_sb[:, m * 128 : (m + 1) * 128],
                        h_sb[:, c0:c1],
                        start=True,
                        stop=True,
                    ).then_inc(s_mm2, 1)

        # tt[0] + piece-0 output cols on DVE; tt[1] + piece-1 cols on Act;
        # piece-2 cols (if any) on Pool.
        @block.vector
        def _(vector):
            vector.wait_ge(s_mm1, 1)
            nc.vector.tensor_copy(
                tt_sb[:, 0:128], tt_ps[:, 0:128]
            ).then_inc(s_tt, 1)
            c0, c1 = pieces[0]
            for m in range(MC):
                vector.wait_ge(s_mm2, N_COPY * m + pos[0])
                nc.vector.tensor_copy(
                    o_sb[m][:, c0:c1], o_ps[m][:, c0:c1]
                ).then_inc(s_oc[m], 1)

        @block.scalar
        def _(scalar):
            scalar.wait_ge(s_mm1, 2)
            nc.scalar.copy(tt_sb[:, 128:256], tt_ps[:, 128:256]).then_inc(
                s_tt, 1
            )
            c0, c1 = pieces[1]
            for m in range(MC):
                scalar.wait_ge(s_mm2, N_COPY * m + pos[1])
                nc.scalar.copy(
                    o_sb[m][:, c0:c1], o_ps[m][:, c0:c1]
                ).then_inc(s_oc[m], 1)

        if N_COPY > 2:

            @block.gpsimd
            def _(gpsimd):
                c0, c1 = pieces[2]
                for m in range(MC):
                    gpsimd.wait_ge(s_mm2, N_COPY * m + pos[2])
                    nc.gpsimd.tensor_copy(
                        o_sb[m][:, c0:c1], o_ps[m][:, c0:c1]
                    ).then_inc(s_oc[m], 1)

    # Drop the unused const-AP memsets bass emits unconditionally in its
    # preamble (the BIR verifier itself flags them as having no reader);
    # they serialize ~380ns on Pool ahead of the startup barrier.
    b0 = nc.m.functions[0].blocks[0]
    b0.instructions = [
        i
        for i in b0.instructions
        if not (
            type(i).__name__ == "InstMemset"
            and str(getattr(i.outs[0], "memref", "")).startswith("const-")
        )
    ]
    # Drop the exit all-engine-barrier semaphore ops: the SP stream already
    # ends on wait_ge(s_out) after the last output DMA receipt, so every
    # output byte is in HBM before any engine halts; the cross-engine
    # EVSEM handshake only aligns halt times (~260ns).
    for b in nc.m.functions[0].blocks:
        if str(getattr(b, "name", "")).endswith("_end"):
            b.instructions = [
                i
                for i in b.instructions
                if not (
                    type(i).__name__ == "InstEventSemaphore"
                    and str(i.name).startswith("aeb_barrier")
                )
            ]
    # Drop the startup all-engine barrier as well (~450ns): every
    # cross-engine dependency in this kernel is carried by its own
    # semaphores (DMA sems gate all consumers), and each engine's register
    # preamble precedes its own work within its own stream.
    b0.instructions = [
        i for i in b0.instructions if not str(i.name).startswith("barrier_")
    ]
    # Drop the per-engine zero/bounds-check RegisterMoves and the startup
    # Drains: this kernel's DMAs are all static (no dynamic-AP bounds checks,
    # nothing reads SP_zero/bcreg*), and nothing is in flight at entry for a
    # Drain to flush. Saves ~250ns of serial preamble on every engine.
    if STRIP_PREAMBLE:
        b0.instructions = [
            i
            for i in b0.instructions
            if type(i).__name__ not in ("InstRegisterMove", "InstDrain")
        ]

    _cache[key] = nc
    return nc


def _pack_inputs(hidden_states, W_seq, hidden_embeddings, cp_weight):
    """Build the per-core packed SBUF images (bf16).

    xw image:   cols [0,256)          w[p, k*64+r]            = W_seq[r, k*128+p]
                cols [256+m*512, ...) xt[p, m*512+k*128+n]    = X[c*256+m*128+n, k*128+p]
    h image:    h[r, j]               = (hidden_embeddings * cp)[j, r]
    """
    import ml_dtypes

    bf16 = ml_dtypes.bfloat16
    X = hidden_states.reshape(ROWS, H)
    xt = (
        X.astype(bf16)
        .reshape(N_CORES, MC, 128, KC, 128)  # [c, m, n, k, p]
        .transpose(0, 4, 1, 3, 2)            # [c, p, m, k, n]
        .reshape(N_CORES, 128, MC * XT_COLS)
    )
    w = (
        W_seq.astype(np.float32)
        .reshape(R, KC, 128)                 # [r, k, p]
        .transpose(2, 1, 0)                  # [p, k, r]
        .reshape(128, W_COLS)
        .astype(bf16)
    )
    xw = np.ascontiguousarray(
        np.concatenate([np.broadcast_to(w, (N_CORES, 128, W_COLS)), xt], axis=2)
    )                                        # [c, 128, IMG_COLS]
    h = np.ascontiguousarray(
        (hidden_embeddings * cp_weight[0][None, :]).T.astype(bf16)
    )                                        # [64, 512]
    return xw, h


def _run_device(xw, h, trace=False, **run_kwargs):
    global LAST_RESULT
    from concourse.bass_utils import run_bass_kernel_spmd

    nc = _get_nc()
    in_maps = [{"xw": xw[c], "h": h} for c in range(N_CORES)]
    res = run_bass_kernel_spmd(
        nc, in_maps, core_ids=list(range(N_CORES)), trace=trace, **run_kwargs
    )
    LAST_RESULT = res
    return np.concatenate(
        [
            np.concatenate(
                [
                    np.asarray(res.results[c][f"out{m}"]).astype(np.float32)
                    for m in range(MC)
                ],
                axis=0,
            )
            for c in range(N_CORES)
        ],
        axis=0,
    )  # [2048, 512] f32


def _host_reference(hidden_states, W_seq, hidden_embeddings, cp_weight):
    """Pure-numpy fallback (correct, host-only)."""
    hid_fac = hidden_embeddings * cp_weight[0][None, :]
    X = hidden_states.reshape(ROWS, H)
    return (X @ W_seq.T @ hid_fac.T).astype(np.float32)


def kernel(hidden_states, all_indices, W_seq, hidden_embeddings, cp_weight,
           trace=False, **run_kwargs):
    hidden_states = np.asarray(hidden_states, dtype=np.float32)
    W_seq = np.asarray(W_seq, dtype=np.float32)
    hidden_embeddings = np.asarray(hidden_embeddings, dtype=np.float32)
    cp_weight = np.asarray(cp_weight, dtype=np.float32)
    all_indices = np.asarray(all_indices)

    try:
        xw, h = _pack_inputs(hidden_states, W_seq, hidden_embeddings, cp_weight)
        Y = _run_device(xw, h, trace=trace, **run_kwargs)
    except Exception as e:  # device unavailable/wedged: stay correct on host
        import traceback

        traceback.print_exc()
        print(f"kernel: device path failed ({type(e).__name__}); "
              "falling back to host compute")
        Y = _host_reference(hidden_states, W_seq, hidden_embeddings, cp_weight)

    P = Y.reshape(B, S, H)

    n = all_indices.shape[0]
    si = all_indices[:, 0].astype(np.int64)
    hi = all_indices[:, 1].astype(np.int64)
    flat = si * H + hi
    if n == S * H and np.array_equal(flat, np.arange(S * H, dtype=np.int64)):
        return P  # cartesian-product indices: the gather is the identity
    return P.reshape(B, S * H)[:, flat].reshape(B, S, n // S)



# revision 22
# speedup vs baseline: 1.3974x; 1.3974x over previous
"""Trainium2 Bass kernel for nn_CPCircuitLayer_63350767616542 (embedding_lookup).

Reference math:
    seq_emb = einsum("bsh,rh->bsr", hidden_states, W_seq)          # [B,S,R]
    hid_fac = hidden_embeddings * cp_weight[0][None, :]            # [H,R]
    out[b,n] = sum_r seq_emb[b, si[n], r] * hid_fac[hi[n], r]      # [B,N]
    return out.reshape(B, S, N // S)

all_indices is the row-major cartesian product of (seq_idx, hidden_idx), so the
gather is the identity and the whole layer collapses to a two-matmul chain:
    P = hidden_states @ W_seq.T @ hid_fac.T                        # [B,S,H]
A host-side fallback gather handles any non-cartesian index list.

Sharding: flatten (B,S) -> 2048 rows, shard rows across the 8 cores (256 rows
per core, data-parallel, no collectives). Each core splits its rows into two
128-row chunks m and computes, per chunk,
    tt[m] = W_seq @ X_m^T            ([64, 128], 4 accumulating k-matmuls)
    O_m   = tt[m]^T @ hid_fac^T      ([128, 512], emitted in column pieces)
with bf16 operands and bf16 output (host converts back to f32).

Device schedule (raw bass, hand-scheduled):
    SP:   the three input DMAs: [W|X0], [X1], [h].
    Pool: zeroes the kv ctx-index tile, then PREPARES a kv_writeback DMA for
          the whole [256,512] output (descriptor gen runs here, ~1us, fully
          off the critical path), copies its share of output columns, and
          finally TRIGGERS the prepared writeback once every output piece is
          in SBUF. The trigger only pays DMA-engine transfer + sem time - the
          HWDGE/DGE latency (~1.3us) that a dma_start would pay after the
          last copy is already spent during startup.
    PE:   mm1[m] as [W|Xm] lands; mm2 column pieces per m once tt[m] is
          staged in SBUF (bf16) and h has landed.
    DVE:  tt[0] PSUM->SBUF(bf16), then its share of output-column copies.
    Act:  tt[1] PSUM->SBUF(bf16), then its share of output-column copies.
"""

import os

import numpy as np

B, S, H, R = 2, 1024, 512, 64
N_CORES = 8
ROWS = B * S                      # 2048 flattened rows
RPC = ROWS // N_CORES             # 256 rows per core
KC = H // 128                     # 4 contraction chunks of 128
MC = RPC // 128                   # 2 output row chunks of 128
W_COLS = KC * R                   # 256 cols of the packed W image
XT_COLS = KC * 128                # 512 xt cols per row chunk
IMG_COLS = W_COLS + MC * XT_COLS  # 1280 cols of the packed [W|X0|X1] image
# The X1 gather's index tile is [128,8] int16 built by one iota
# (val = p + 16c, p in [0,128)); only rows 0..15 index real transfers, but
# every row is bounds-checked against the source, so the DRAM image carries
# 112 padding rows to keep the check happy.
IMG_ROWS = 240

# Output-copy column split: per-m piece lists "ENG:cols,..." separated by
# ";" (m0;m1). Each list sums to 512; emission order = list order.
PIECES = os.environ.get(
    "BASS_PIECES", "ACT:168,DVE:200,POOL:144;ACT:176,DVE:184,POOL:152")
# Engine for the tt[1] PSUM->SBUF stage ("DVE" or "ACT"); tt[0] is on DVE.
TT1_ENG = os.environ.get("BASS_TT1", "DVE")
# kv-writeback output (prepare early / trigger late) vs plain HWDGE dma_start
OUT_MODE = os.environ.get("BASS_OUT_MODE", "kv")
# X1 input DMA: "swdge" = Pool gather prepared early + triggered right after
# the [W|X0] transfer (skips the second HWDGE gen + DGE delay), or "hwdge".
D2_MODE = os.environ.get("BASS_D2", "swdge")
STRIP_PREAMBLE = os.environ.get("BASS_STRIP_PREAMBLE", "1") == "1"

_cache = {}
LAST_RESULT = None                # BassKernelResults of the most recent run


def _parse_pieces():
    per_m = []
    for mpart in PIECES.split(";"):
        lst = []
        for part in mpart.split(","):
            eng, _, cols = part.partition(":")
            cols = int(cols)
            if cols > 0:
                lst.append((eng.strip().upper(), cols))
        assert sum(c for _, c in lst) == H, PIECES
        per_m.append(lst)
    assert len(per_m) == MC, PIECES
    return per_m


def _get_nc():
    key = ("nc", PIECES, TT1_ENG, OUT_MODE, D2_MODE, STRIP_PREAMBLE)
    if key in _cache:
        return _cache[key]

    import concourse.bass as bass
    import concourse.mybir as mybir

    f32 = mybir.dt.float32
    bf16 = mybir.dt.bfloat16
    i32 = mybir.dt.int32
    i16 = mybir.dt.int16

    per_m_pieces = _parse_pieces()

    nc = bass.Bass(
        "TRN2",
        target_bir_lowering=False,
        debug=False,
        num_devices=N_CORES,
    )

    xw_d = nc.dram_tensor("xw", [IMG_ROWS, IMG_COLS], bf16,
                          kind="ExternalInput")
    h_d = nc.dram_tensor("h", [R, H], bf16, kind="ExternalInput")
    out_d = nc.dram_tensor("out", [RPC, H], bf16, kind="ExternalOutput")

    from contextlib import ExitStack

    with ExitStack() as stack:
        ec = stack.enter_context
        xw_sb = ec(nc.sbuf_tensor([128, IMG_COLS], bf16))
        h_sb = ec(nc.sbuf_tensor([R, H], bf16))
        tt_sb = ec(nc.sbuf_tensor([R, MC * 128], bf16))
        o_sb = ec(nc.sbuf_tensor([128, MC * H], bf16))
        idx_sb = ec(nc.sbuf_tensor([128, MC], i32))
        gidx_sb = ec(nc.sbuf_tensor([128, 8], i16))
        tt0_ps = ec(nc.psum_tensor([R, 128], f32))
        tt1_ps = ec(nc.psum_tensor([R, 128], f32))
        o_ps = ec(nc.psum_tensor([128, MC * H], f32))
        (s_d1, s_d2, s_h, s_mm1, s_tt0, s_tt1, s_mm2,
         s_ocd, s_oca, s_ocp, s_prep, s_kv, s_out, s_idx,
         s_gidx, s_p2) = (
            ec(nc.semaphore(n))
            for n in ("s_d1", "s_d2", "s_h", "s_mm1", "s_tt0", "s_tt1",
                      "s_mm2", "s_ocd", "s_oca", "s_ocp", "s_prep",
                      "s_kv", "s_out", "s_idx", "s_gidx", "s_p2"))
        block = ec(nc.Block(no_gpsimd_drain=True))
        oc_sem = {"DVE": s_ocd, "ACT": s_oca, "POOL": s_ocp}
        # (engine, m, c0, c1, global emission position 1-based)
        sched = []
        pos = 0
        for m in range(MC):
            c = 0
            for eng, cols in per_m_pieces[m]:
                pos += 1
                sched.append((eng, m, c, c + cols, pos))
                c += cols

        def w_slice(k):
            return xw_sb[:, k * R : (k + 1) * R]

        def xt_slice(m, k):
            c0 = W_COLS + m * XT_COLS + k * 128
            return xw_sb[:, c0 : c0 + 128]

        @block.sync
        def _(sync):
            d1_cols = W_COLS + XT_COLS
            sync.dma_start(
                xw_sb[:, 0:d1_cols], xw_d.ap()[0:128, 0:d1_cols]
            ).then_inc(s_d1, 16)
            if D2_MODE != "swdge":
                sync.dma_start(
                    xw_sb[:, d1_cols:IMG_COLS],
                    xw_d.ap()[0:128, d1_cols:IMG_COLS],
                ).then_inc(s_d2, 16)
            sync.dma_start(h_sb[:], h_d.ap()).then_inc(s_h, 16)
            if OUT_MODE == "hwdge":
                for m in range(MC):
                    for eng in set(e for e, *_ in sched):
                        n = sum(1 for (e, mm_, *_r) in sched
                                if e == eng and mm_ <= m)
                        sync.wait_ge(oc_sem[eng], n)
                    sync.dma_start(
                        out_d.ap()[m * 128 : (m + 1) * 128, :],
                        o_sb[:, m * H : (m + 1) * H],
                    ).then_inc(s_out, 16)
                sync.wait_ge(s_out, 16 * MC)

        tt_ps = [tt0_ps, tt1_ps]

        @block.tensor
        def _(tensor):
            # NB: keep PE waits as standalone EventSemaphores - waiting holds
            # PE.SEQ, which counts as "busy" for the p-state ramp model, so
            # the clock is already at mid speed when the first matmul lands.
            for m in range(MC):
                tensor.wait_ge(s_d1 if m == 0 else s_d2, 16)
                for k in range(KC):
                    mm = nc.tensor.matmul(
                        tt_ps[m][:],
                        w_slice(k),
                        xt_slice(m, k),
                        start=(k == 0),
                        stop=(k == KC - 1),
                    )
                mm.then_inc(s_mm1, 1)
            tensor.wait_ge(s_h, 16)
            for m in range(MC):
                tensor.wait_ge(s_tt0 if m == 0 else s_tt1, 1)
                for (eng, mm_, c0, c1, p) in sched:
                    if mm_ != m:
                        continue
                    nc.tensor.matmul(
                        o_ps[:, m * H + c0 : m * H + c1],
                        tt_sb[:, m * 128 : (m + 1) * 128],
                        h_sb[:, c0:c1],
                        start=True,
                        stop=True,
                    ).then_inc(s_mm2, 1)

        @block.vector
        def _(vector):
            nc.vector.tensor_copy(
                tt_sb[:, 0:128], tt0_ps[:]
            )._wait_ge(s_mm1, 1).then_inc(s_tt0, 1)
            if TT1_ENG == "DVE":
                nc.vector.tensor_copy(
                    tt_sb[:, 128:256], tt1_ps[:]
                )._wait_ge(s_mm1, 2).then_inc(s_tt1, 1)
            for (eng, m, c0, c1, p) in sched:
                if eng != "DVE":
                    continue
                nc.vector.tensor_copy(
                    o_sb[:, m * H + c0 : m * H + c1],
                    o_ps[:, m * H + c0 : m * H + c1],
                )._wait_ge(s_mm2, p).then_inc(s_ocd, 1)

        @block.scalar
        def _(scalar):
            if TT1_ENG == "ACT":
                nc.scalar.copy(
                    tt_sb[:, 128:256], tt1_ps[:]
                )._wait_ge(s_mm1, 2).then_inc(s_tt1, 1)
            for (eng, m, c0, c1, p) in sched:
                if eng != "ACT":
                    continue
                nc.scalar.copy(
                    o_sb[:, m * H + c0 : m * H + c1],
                    o_ps[:, m * H + c0 : m * H + c1],
                )._wait_ge(s_mm2, p).then_inc(s_oca, 1)

        @block.gpsimd
        def _(gpsimd):
            from concourse import library_config

            if D2_MODE == "swdge":
                # gather indices 0..127 wrapped [16,8]: val = p + 16*c
                nc.gpsimd.iota(
                    gidx_sb[:], pattern=[[16, 8]], base=0,
                    channel_multiplier=1,
                ).then_inc(s_gidx, 1)
            if OUT_MODE == "kv" or D2_MODE == "swdge":
                # kv_writeback + dma_gather live in the attnmlp Q7 library
                # (iota above runs under the boot-default standard library).
                nc.gpsimd.load_library(library_config.attnmlp)
            if D2_MODE == "swdge":
                d1_cols = W_COLS + XT_COLS
                gpsimd.wait_ge(s_gidx, 1)
                nc.gpsimd.dma_gather(
                    xw_sb[:, d1_cols:IMG_COLS].rearrange(
                        "p (o j) -> p o j", o=1),
                    xw_d.ap()[:, d1_cols:IMG_COLS],
                    gidx_sb[:],
                    num_idxs=128,
                    num_idxs_reg=128,
                    elem_size=XT_COLS,
                    elem_step=IMG_COLS,
                    prepare_only=True,
                    sem=s_d2,
                ).then_inc(s_p2, 1)
                nc.gpsimd.trigger_dma(count=1)._wait_ge(s_p2, 1)
            if OUT_MODE == "kv":
                # ctx indices (all zeros) must be in SBUF before the
                # prep reads them; the sem edge orders the two Pool ops.
                nc.gpsimd.memset(idx_sb[:], 0).then_inc(s_idx, 1)
                gpsimd.wait_ge(s_idx, 1)
                # Descriptor gen (~1us on the Q7s) runs here, way before the
                # copies land; the trigger below only fires the DMA engines.
                in_ap = o_sb[:].rearrange("p (b o j) -> p o b j", b=MC, o=1)
                out_ap = out_d.ap().rearrange(
                    "(b p o) j -> b p o j", b=MC, o=1
                )
                nc.gpsimd.kv_writeback(
                    out_ap,
                    in_ap,
                    idx_sb[:],
                    prepare_only=True,
                    sem=s_kv,
                ).then_inc(s_prep, 1)
            for (eng, m, c0, c1, p) in sched:
                if eng != "POOL":
                    continue
                nc.gpsimd.tensor_copy(
                    o_sb[:, m * H + c0 : m * H + c1],
                    o_ps[:, m * H + c0 : m * H + c1],
                )._wait_ge(s_mm2, p).then_inc(s_ocp, 1)
            if OUT_MODE == "kv":
                # <=2 sem waits fit on one instruction: park prep + Pool-copy
                # completion on a standalone event, DVE/Act on the trigger.
                counts = {
                    eng: sum(1 for (e, *_r) in sched if e == eng)
                    for eng in ("DVE", "ACT", "POOL")
                }
                ev = gpsimd.wait_ge(s_prep, 1)
                if counts["POOL"]:
                    ev._wait_ge(s_ocp, counts["POOL"])
                if counts["DVE"]:
                    ev2 = gpsimd.wait_ge(s_ocd, counts["DVE"])
                    if counts["ACT"]:
                        ev2._wait_ge(s_oca, counts["ACT"])
                elif counts["ACT"]:
                    gpsimd.wait_ge(s_oca, counts["ACT"])
                nc.gpsimd.trigger_dma(count=1)
                if os.environ.get("BASS_KV_WAIT", "0") == "1":
                    gpsimd.wait_ge(s_kv, 16)

    # Drop the unused const-AP memsets bass emits unconditionally in its
    # preamble (the BIR verifier itself flags them as having no reader);
    # they serialize ~380ns on Pool ahead of the startup barrier.
    b0 = nc.m.functions[0].blocks[0]
    b0.instructions = [
        i
        for i in b0.instructions
        if not (
            type(i).__name__ == "InstMemset"
            and str(getattr(i.outs[0], "memref", "")).startswith("const-")
        )
    ]
    # Drop the exit all-engine-barrier semaphore ops: the kv completion wait
    # (or s_out wait in hwdge mode) already keeps the program alive until
    # every output byte is in HBM; the cross-engine EVSEM handshake only
    # aligns halt times (~260ns).
    for b in nc.m.functions[0].blocks:
        if str(getattr(b, "name", "")).endswith("_end"):
            b.instructions = [
                i
                for i in b.instructions
                if not (
                    type(i).__name__ == "InstEventSemaphore"
                    and str(i.name).startswith("aeb_barrier")
                )
            ]
    # Drop the startup all-engine barrier as well (~450ns): every
    # cross-engine dependency in this kernel is carried by its own
    # semaphores (DMA sems gate all consumers), and each engine's register
    # preamble precedes its own work within its own stream.
    b0.instructions = [
        i for i in b0.instructions if not str(i.name).startswith("barrier_")
    ]
    # Drop the per-engine zero/bounds-check RegisterMoves and the startup
    # Drains: this kernel's DMAs are all static (no dynamic-AP bounds checks,
    # nothing reads SP_zero/bcreg*), and nothing is in flight at entry for a
    # Drain to flush. Saves ~250ns of serial preamble on every engine.
    if STRIP_PREAMBLE:
        def _is_preamble(i):
            if type(i).__name__ == "InstDrain":
                return True
            if type(i).__name__ != "InstRegisterMove":
                return False
            ref = str(getattr(i.outs[0], "regref", ""))
            return ("_zero" in ref or "_bcreg" in ref or "_monotonic" in ref)

        b0.instructions = [i for i in b0.instructions if not _is_preamble(i)]

    _cache[key] = nc
    return nc


def _pack_inputs(hidden_states, W_seq, hidden_embeddings, cp_weight):
    """Build the per-core packed SBUF images (bf16).

    xw image:   cols [0,256)          w[p, k*64+r]            = W_seq[r, k*128+p]
                cols [256+m*512, ...) xt[p, m*512+k*128+n]    = X[c*256+m*128+n, k*128+p]
    h image:    h[r, j]               = (hidden_embeddings * cp)[j, r]
    """
    import ml_dtypes

    bf16 = ml_dtypes.bfloat16
    X = hidden_states.reshape(ROWS, H)
    xt = (
        X.astype(bf16)
        .reshape(N_CORES, MC, 128, KC, 128)  # [c, m, n, k, p]
        .transpose(0, 4, 1, 3, 2)            # [c, p, m, k, n]
        .reshape(N_CORES, 128, MC * XT_COLS)
    )
    w = (
        W_seq.astype(np.float32)
        .reshape(R, KC, 128)                 # [r, k, p]
        .transpose(2, 1, 0)                  # [p, k, r]
        .reshape(128, W_COLS)
        .astype(bf16)
    )
    xw = np.concatenate(
        [np.broadcast_to(w, (N_CORES, 128, W_COLS)), xt], axis=2
    )                                        # [c, 128, IMG_COLS]
    pad = np.zeros((N_CORES, IMG_ROWS - 128, IMG_COLS), dtype=bf16)
    xw = np.ascontiguousarray(np.concatenate([xw, pad], axis=1))
    h = np.ascontiguousarray(
        (hidden_embeddings * cp_weight[0][None, :]).T.astype(bf16)
    )                                        # [64, 512]
    return xw, h


def _run_device(xw, h, trace=False, **run_kwargs):
    global LAST_RESULT
    from concourse.bass_utils import run_bass_kernel_spmd

    nc = _get_nc()
    in_maps = [{"xw": xw[c], "h": h} for c in range(N_CORES)]
    res = run_bass_kernel_spmd(
        nc, in_maps, core_ids=list(range(N_CORES)), trace=trace, **run_kwargs
    )
    LAST_RESULT = res
    return np.concatenate(
        [
            np.asarray(res.results[c]["out"]).astype(np.float32)
            for c in range(N_CORES)
        ],
        axis=0,
    )  # [2048, 512] f32


def _host_reference(hidden_states, W_seq, hidden_embeddings, cp_weight):
    """Pure-numpy fallback (correct, host-only)."""
    hid_fac = hidden_embeddings * cp_weight[0][None, :]
    X = hidden_states.reshape(ROWS, H)
    return (X @ W_seq.T @ hid_fac.T).astype(np.float32)


def kernel(hidden_states, all_indices, W_seq, hidden_embeddings, cp_weight,
           trace=False, **run_kwargs):
    hidden_states = np.asarray(hidden_states, dtype=np.float32)
    W_seq = np.asarray(W_seq, dtype=np.float32)
    hidden_embeddings = np.asarray(hidden_embeddings, dtype=np.float32)
    cp_weight = np.asarray(cp_weight, dtype=np.float32)
    all_indices = np.asarray(all_indices)

    try:
        xw, h = _pack_inputs(hidden_states, W_seq, hidden_embeddings, cp_weight)
        Y = _run_device(xw, h, trace=trace, **run_kwargs)
    except Exception as e:  # device unavailable/wedged: stay correct on host
        import traceback

        traceback.print_exc()
        print(f"kernel: device path failed ({type(e).__name__}); "
              "falling back to host compute")
        Y = _host_reference(hidden_states, W_seq, hidden_embeddings, cp_weight)

    P = Y.reshape(B, S, H)

    n = all_indices.shape[0]
    si = all_indices[:, 0].astype(np.int64)
    hi = all_indices[:, 1].astype(np.int64)
    flat = si * H + hi
    if n == S * H and np.array_equal(flat, np.arange(S * H, dtype=np.int64)):
        return P  # cartesian-product indices: the gather is the identity
    return P.reshape(B, S * H)[:, flat].reshape(B, S, n // S)
